# revision 40
# baseline (speedup 1.0000x reference)
"""BoundaryLoss Trainium2 kernel (8 NeuronCores, SPMD, strip-replicated).

Layout: core c owns output column block [128c, 128c+128). The host hands
each core a strip of every input row covering its block plus a margin of
w columns on each side (w = bucketed max in-row nearest-background
distance, measured exactly on the host as in the previous revision).
Row-local EDT distances never exceed w at the central columns, so each
core can run the full row pass locally — no AllToAll at all, which in the
prior revision serialized ~70us of collective latency ahead of the column
pass.

Pipeline (per core):
  1. Row pass on [128, 8*W] fp16 strips (W = 128+2w; partition p, block b
     holds image row 128b+p). One forward + one reverse
     tensor_tensor_scan per image; the scan chains across block
     boundaries, but any carried-in state reaches a central column with
     value > w and so never wins (margin absorbs it).
  2. PE-transpose the central 128 columns of each block (g, fp16), square
     on the PSUM->SBUF evacuation (ACT), assembling g2^T [128 cols, 1024
     rows] directly — all overlapped with the other image's row pass.
  3. Column min-plus D2[j,i] = min_dd (dd^2 + g2T[j, i+dd]) over
     |dd| <= w on DVE in fp16 when w <= 44 (integers <= 2048 are fp16-
     exact; candidates in (2048, 4096] round by <= 1, a <= 0.05% error),
     f32 (STT pairs) otherwise. Odd shifts read a one-element-shifted
     copy to keep 4-byte alignment for the DVE 2x mode.
  4. Per-image global max via one small AllReduce (a dummy AllReduce at
     t=0 absorbs this runtime's ~55us first-collective barrier under the
     compute), then a short fp16 tail: masks compare unnormalized
     d = sqrt(D2) against 0.1*(max+1e-6), diff/abs/masked partial sums
     with fused accumulate; host sums the 8 partial pairs.
"""
import os
import sys

import numpy as np

for _p in ("/opt/trn_rl_repo", "/root/.axon_site/_ro/trn_rl_repo"):
    if os.path.isdir(_p) and _p not in sys.path:
        sys.path.append(_p)

import concourse.bacc as bacc
import concourse.tile as tile
from concourse import mybir
from concourse.bass_utils import run_bass_kernel_spmd

F32 = mybir.dt.float32
FP16 = mybir.dt.float16
I32 = mybir.dt.int32
AF = mybir.ActivationFunctionType
ALU = mybir.AluOpType
AX = mybir.AxisListType

H = 1024          # image height/width
P = 128           # partitions / rows per block / cols per core block
NB = 8            # row blocks per strip (H / P)
NCORES = 8
BIG = 1.0e4
INF = 1.0e9       # f32 sentinel
HINF = 60000.0    # fp16 sentinel (fp16 max normal is 65504)
FP16_WMAX = 44    # fp16 col pass iff w <= 44 (g^2, dd^2 <= 1936 exact)

_BUCKETS = (8, 10, 12, 14, 16, 18, 20, 22, 24, 26, 28, 32, 36, 40, 44,
            48, 56, 64, 80, 96, 128, 160, 192, 256, 320)


def _col_pass(tc, m, w, gTp, gB, persist, work):
    """Windowed min-plus; returns acc tile [P, H] (fp16 or f32).

    acc[j, i] = min_{|dd| <= w} (dd^2 + gTp[j, w + i + dd]); gTp is
    INF-padded by w on both sides. Entirely on DVE (tensor ops are
    rejected on Pool in this compiler build).
    """
    nc = tc.nc
    use16 = gB is not None

    if use16:
        def shifted(off):  # AP of width H at element offset `off` of gTp
            if off % 2 == 0:
                return gTp[:, off:off + H]
            return gB[:, off - 1:off - 1 + H]
    else:
        def shifted(off):
            return gTp[:, off:off + H]

    acc = persist.tile([P, H], FP16 if use16 else F32, tag=f"acc{m}")
    # Plain TT gets the DVE 2x mode for 16-bit and single-src TS gets 4x,
    # while the fused STT has no fast uop — so for fp16 a 3-op pairwise
    # form beats 2 STTs per dd. dd=1 folds the d=0 term.
    if use16:
        for dd in range(1, w + 1):
            tmp = work.tile([P, H], FP16, tag=f"pm{m}_{dd % 3}")
            nc.vector.tensor_tensor(tmp[:], shifted(w + dd), shifted(w - dd),
                                    ALU.min)
            nc.vector.tensor_scalar_add(tmp[:], tmp[:], float(dd * dd))
            nc.vector.tensor_tensor(
                acc[:], shifted(w) if dd == 1 else acc[:], tmp[:], ALU.min)
    else:
        for dd in range(1, w + 1):
            c = float(dd * dd)
            nc.vector.scalar_tensor_tensor(
                acc[:], shifted(w + dd), c,
                shifted(w) if dd == 1 else acc[:], ALU.add, ALU.min)
            nc.vector.scalar_tensor_tensor(
                acc[:], shifted(w - dd), c, acc[:], ALU.add, ALU.min)
    return acc


def _body(tc, w_gt, w_pred, srcs, partials):
    nc = tc.nc
    rg = [list(range(NCORES))]
    ws = (w_gt, w_pred)
    use16s = tuple(w <= FP16_WMAX for w in ws)

    with tc.tile_pool(name="const", bufs=1) as const, \
         tc.tile_pool(name="work", bufs=2) as work, \
         tc.tile_pool(name="persist", bufs=1) as persist, \
         tc.tile_pool(name="tail", bufs=1) as tail, \
         tc.tile_pool(name="ps", bufs=1, space="PSUM") as ps, \
         tc.tile_pool(name="dram", bufs=1, space="DRAM") as dram:

        # ---- input DMA ----
        # Both images ride one concatenated strip tensor, split across the
        # two HWDGE queues (SP and ACT) — DMA trigger instructions cost
        # ~0.7us each on the issuing queue.
        wds = [NB * (P + 2 * w) for w in ws]
        WD = sum(wds)
        st = persist.tile([P, WD], FP16, tag="st")
        for q in range(4):
            eng = nc.sync if q % 2 == 0 else nc.scalar
            eng.dma_start(st[q * 32:(q + 1) * 32, :],
                          srcs[q * 32:(q + 1) * 32, :])

        # ---- collective plumbing ----
        # Observed CC-stream behavior on this runtime: an automatic
        # barrier runs ~21.3us -> ~56-70us whether or not a collective has
        # been triggered yet; the first collective starts at
        # max(barrier_end, trigger) + ~11.5us and runs ~10us when its
        # trigger predates the barrier end but ~21us when triggered after
        # it; a collective_compute instruction blocks its issuing engine
        # queue until the collective completes. So: per-image AllReduces,
        # the first triggered right after the (shorter) gt column pass
        # ~48us — early enough for the fast path — doing useful work
        # instead of a dummy warm-up; the second queues behind it.
        ar_ins = [dram.tile([1, 1], F32, name=f"ari{m}", tag=f"ari{m}")
                  for m in range(2)]
        ar_outs = [nc.dram_tensor(f"ar_out_sh{m}", [1, NCORES], F32,
                                  addr_space="Shared") for m in range(2)]

        # ---- constants (DVE is idle while the strips stream in) ----
        io = const.tile([P, P], I32)
        nc.gpsimd.iota(io[:], [[1, P]], base=0, channel_multiplier=-1)
        ident = const.tile([P, P], F32)
        nc.vector.tensor_scalar(ident[:], io[:], 0, None, ALU.is_equal)
        identh = const.tile([P, P], FP16)
        nc.scalar.copy(identh[:], ident[:])
        ones1 = const.tile([1, P], F32)
        nc.vector.memset(ones1[:], 1.0)
        onesc = const.tile([P, 1], F32)
        nc.vector.memset(onesc[:], 1.0)
        onesh = const.tile([P, WD], FP16)
        nc.vector.memset(onesh[:], 1.0)

        # ================= phase 1: row pass =================
        # One op each over the whole concatenation: the scans chain across
        # block/image boundaries, but carried-in state reaches any central
        # column with value > that image's margin and never wins.
        # foreground -> HINF, background -> 0. Host pre-scales inputs by
        # 1e30 (saturating fp16) so `> 0` is the fg test for both images
        # and fp16 underflow cannot flip tiny positives.
        z = work.tile([P, WD], FP16, tag="z")
        nc.vector.tensor_scalar(z[:], st[:], 0.0, HINF, ALU.is_gt, ALU.mult)
        dl = work.tile([P, WD], FP16, tag="dl")
        nc.vector.tensor_tensor_scan(dl[:], onesh[:], z[:], INF,
                                     ALU.add, ALU.min)
        dr = work.tile([P, WD], FP16, tag="dr")
        nc.vector.tensor_tensor_scan(dr[:, ::-1], onesh[:], z[:, ::-1], INF,
                                     ALU.add, ALU.min)
        g = work.tile([P, WD], FP16, tag="g")
        nc.vector.tensor_tensor(g[:], dl[:], dr[:], ALU.min)

        # ============ phase 2: transpose + square into g2^T ============
        gTps = []
        gBs = []
        for m in range(2):
            w = ws[m]
            use16 = use16s[m]
            dt = FP16 if use16 else F32
            inf = HINF if use16 else INF
            gw = H + 2 * w
            gTp = persist.tile([P, gw], dt, tag=f"gtp{m}")
            nc.vector.memset(gTp[:, :w], inf)
            nc.vector.memset(gTp[:, w + H:], inf)
            W = P + 2 * w
            base = m * wds[0]
            for b in range(NB):
                pt = ps.tile([P, P], FP16, tag="pt", bufs=4)
                off = base + b * W + w
                nc.tensor.transpose(pt[:], g[:, off:off + P], identh[:])
                nc.scalar.activation(gTp[:, w + b * P:w + (b + 1) * P], pt[:],
                                     AF.Square)
            if use16:
                # odd shifts read a one-element-shifted copy so the AP
                # stays 4-byte-aligned for the DVE 2x fp16 mode
                gB = persist.tile([P, gw], FP16, tag=f"gb{m}")
                nc.scalar.copy(gB[:, :gw - 1], gTp[:, 1:])
                nc.vector.memset(gB[:, gw - 1:], inf)
            else:
                gB = None
            gTps.append(gTp)
            gBs.append(gB)

        # ====== phase 3: column min-plus + per-image max + AllReduce ======
        # DVE queue order: col0, mx0, mxr0, col1, mx1, mxr1 — the tiny
        # partition-max of image 0 rides between the passes so AllReduce#0
        # triggers at ~48us, well before the CC barrier ends.
        ys = []
        mxrs = []
        accs = []
        for m in range(2):
            acc = _col_pass(tc, m, ws[m], gTps[m], gBs[m], persist, work)
            accs.append(acc)
            mxp = tail.tile([P, 1], F32, tag=f"mxp{m}")
            nc.vector.reduce_max(mxp[:], acc[:], axis=AX.X)
            pmx = ps.tile([1, P], F32, tag="pmx")
            nc.tensor.transpose(pmx[:], mxp[:], ident[:])
            mxr = tail.tile([1, 1], F32, tag=f"mxr{m}")
            nc.vector.reduce_max(mxr[:], pmx[:], axis=AX.X)
            mxrs.append(mxr)
            nc.sync.dma_start(ar_ins[m][0:1, 0:1], mxr[:])
            nc.gpsimd.collective_compute(
                "AllGather", ALU.bypass, replica_groups=rg,
                ins=[ar_ins[m][:, :].opt()], outs=[ar_outs[m][:, :].opt()])
            # unnormalized distances, precomputed before AllReduce#1 ends
            y = persist.tile([P, H], FP16, tag=f"y{m}")
            nc.scalar.activation(y[:], acc[:], AF.Sqrt)
            ys.append(y)

        # ============ phase 4: per-image inv = 1/(sqrt(max)+1e-6) ============
        # Image 0's chain (and a0 below) runs while AllReduce#1 is in
        # flight; image 1's is the post-collective critical path. The DVE
        # bits are emitted after both column passes so the in-order DVE
        # queue never stalls on a collective result mid-compute.
        def inv_chain(m, guard=None):
            # Fetched on the ACT HWDGE queue: on the SP queue this trigger
            # (gated on the collective's completion) gets scheduled ahead
            # of AllGather#1's input staging and head-of-line blocks it.
            gmx8 = tail.tile([1, NCORES], F32, tag=f"gmx8{m}")
            nc.scalar.dma_start(gmx8[:], ar_outs[m][0:1, :])
            gmx = tail.tile([1, 1], F32, tag=f"gmx{m}")
            if guard is None:
                nc.vector.reduce_max(gmx[:], gmx8[:], axis=AX.X)
            else:
                # The scheduler also hoists DVE ops above earlier-created
                # ones; a collective-gated op hoisted above the image-1
                # column pass would head-of-line block the DVE queue (and
                # with it AllGather#1's trigger). op0=bypass both ignores
                # the `guard` scalar AND records a read of it, pinning
                # this op (and its dependents) behind the column pass.
                t4 = tail.tile([1, 4], F32, tag="t4")
                nc.vector.scalar_tensor_tensor(
                    t4[:], gmx8[:, 0:4], guard, gmx8[:, 4:8],
                    ALU.bypass, ALU.max)
                t2 = tail.tile([1, 2], F32, tag="t2")
                nc.vector.tensor_tensor(t2[:], t4[:, 0:2], t4[:, 2:4],
                                        ALU.max)
                nc.vector.tensor_tensor(gmx[:], t2[:, 0:1], t2[:, 1:2],
                                        ALU.max)
            msq = tail.tile([1, 1], F32, tag=f"msq{m}")
            nc.scalar.activation(msq[:], gmx[:], AF.Sqrt)
            s1 = tail.tile([1, 1], F32, tag=f"s1{m}")
            nc.vector.tensor_scalar_add(s1[:], msq[:], 1e-6)
            nc.vector.reciprocal(s1[:], s1[:])
            pb = ps.tile([P, 1], F32, tag="pb")
            nc.tensor.matmul(pb[:], ones1[:], s1[:])
            invb = tail.tile([P, 1], F32, tag=f"invb{m}")
            nc.scalar.copy(invb[:], pb[:])
            return invb

        # ================= phase 5: normalize + masked mean =================
        # Image 0's inv chain and a0 (on ACT; scale is a per-partition AP)
        # overlap AllReduce#1; image 1's chain and a1 (DVE) are the
        # post-collective critical path.
        invb0 = inv_chain(0, guard=mxrs[1][0:1, 0:1])
        a0 = tail.tile([P, H], FP16, tag="a0")
        nc.scalar.activation(a0[:], ys[0][:], AF.Copy, scale=invb0[:, 0:1])
        invb1 = inv_chain(1)
        a1 = tail.tile([P, H], FP16, tag="a1")
        nc.vector.tensor_scalar(a1[:], ys[1][:], invb1[:, 0:1], None,
                                ALU.mult)
        # (a0 < 0.1) | (a1 < 0.1)  ==  min(a0, a1) < 0.1; the masked-|diff|
        # sum and the mask count ride the ACT accumulators (|mk*df| =
        # mk*|df|), keeping only 5 ops on the post-collective DVE path.
        mk = tail.tile([P, H], FP16, tag="mk")
        nc.vector.tensor_tensor(mk[:], a0[:], a1[:], ALU.min)
        nc.vector.tensor_scalar(mk[:], mk[:], 0.1, None, ALU.is_lt)
        df = tail.tile([P, H], FP16, tag="df")
        nc.vector.tensor_tensor(df[:], a0[:], a1[:], ALU.subtract)
        nc.vector.tensor_tensor(df[:], df[:], mk[:], ALU.mult)
        # per-partition partial sums go out as [128, 2]; the host finishes
        # the 256-element reduction (frees the PE matmul + PSUM-evacuate
        # hops from the post-collective critical path)
        s12 = tail.tile([P, 2], F32, tag="s12")
        cnt = tail.tile([P, H], FP16, tag="cnt")
        nc.scalar.activation(cnt[:], mk[:], AF.Copy, accum_out=s12[:, 1:2])
        da = tail.tile([P, H], FP16, tag="da")
        nc.scalar.activation(da[:], df[:], AF.Abs, accum_out=s12[:, 0:1])
        nc.sync.dma_start(partials[:, :], s12[:])


def _build(w_gt, w_pred):
    nc = bacc.Bacc("TRN2", target_bir_lowering=False, debug=False,
                   num_devices=NCORES)
    wd = NB * (P + 2 * w_gt) + NB * (P + 2 * w_pred)
    st = nc.dram_tensor("st", [P, wd], FP16, kind="ExternalInput")
    partials = nc.dram_tensor("partials", [P, 2], F32, kind="ExternalOutput")
    with tile.TileContext(nc) as tc:
        _body(tc, w_gt, w_pred, st, partials)
    nc.compile()
    return nc


_PROGRAMS = {}


def _program(*key):
    if key not in _PROGRAMS:
        _PROGRAMS[key] = _build(*key)
    return _PROGRAMS[key]


def _row_gmax(fg):
    """Max over pixels of the in-row distance to the nearest background
    pixel (clamped to BIG). This equals the exact column-pass window bound."""
    idx = np.arange(fg.shape[1], dtype=np.float64)
    zero = ~fg
    left = np.maximum.accumulate(np.where(zero, idx, -np.inf), axis=1)
    right = np.minimum.accumulate(np.where(zero, idx, np.inf)[:, ::-1],
                                  axis=1)[:, ::-1]
    g = np.minimum(np.minimum(idx - left, right - idx), BIG)
    return float(g.max())


def _bucket(gmax):
    need = min(int(np.ceil(gmax)), H - 1)
    for b in _BUCKETS:
        if b >= need:
            return b
    raise NotImplementedError(
        f"row gmax {gmax} exceeds the supported strip margin {_BUCKETS[-1]}")


def _strips(img, w):
    """Per-core fp16 strips [128, 8*(128+2w)]: strip[c][p, b*(128+2w)+q] =
    scaled img[128*b + p, 128*c - w + q], fg-padded outside the image."""
    x = np.asarray(img, np.float32) * 1e30
    pad = np.full((H, w), np.float32(1e30))
    xp = np.concatenate([pad, x, pad], axis=1)
    W = P + 2 * w
    out = []
    for c in range(NCORES):
        b = xp[:, c * P:c * P + W].astype(np.float16)
        out.append(np.ascontiguousarray(
            b.reshape(NB, P, W).transpose(1, 0, 2).reshape(P, NB * W)))
    return out


def _run(pred, gt, trace=False):
    pred = np.ascontiguousarray(np.asarray(pred), dtype=np.float32)
    gt = np.ascontiguousarray(np.asarray(gt), dtype=np.float32)
    assert pred.shape == (H, H) and gt.shape == (H, H)
    w_gt = _bucket(_row_gmax(gt != 0))
    w_pred = _bucket(_row_gmax(pred > 0))
    nc = _program(w_gt, w_pred)
    sg = _strips(gt, w_gt)
    sp = _strips(pred, w_pred)
    in_maps = [{"st": np.ascontiguousarray(
        np.concatenate([sg[c], sp[c]], axis=1))} for c in range(NCORES)]
    res = run_bass_kernel_spmd(nc, in_maps, list(range(NCORES)), trace=trace)
    tot = np.zeros(2, np.float64)
    for r in res.results:
        tot += np.asarray(r["partials"], np.float64).reshape(P, 2).sum(0)
    loss = np.float32(tot[0] / max(tot[1], 1.0))
    return loss, res


def kernel(pred, gt):
    loss, _ = _run(pred, gt)
    return loss


# revision 47
# speedup vs baseline: 1.0050x; 1.0050x over previous
"""BoundaryLoss Trainium2 kernel (8 NeuronCores, SPMD, strip-replicated).

Layout: core c owns output column block [128c, 128c+128). The host hands
each core a strip of every input row covering its block plus a margin of
w columns on each side (w = bucketed max in-row nearest-background
distance, measured exactly on the host as in the previous revision).
Row-local EDT distances never exceed w at the central columns, so each
core can run the full row pass locally — no AllToAll at all, which in the
prior revision serialized ~70us of collective latency ahead of the column
pass.

Pipeline (per core):
  1. Row pass on [128, 8*W] fp16 strips (W = 128+2w; partition p, block b
     holds image row 128b+p). One forward + one reverse
     tensor_tensor_scan per image; the scan chains across block
     boundaries, but any carried-in state reaches a central column with
     value > w and so never wins (margin absorbs it).
  2. PE-transpose the central 128 columns of each block (g, fp16), square
     on the PSUM->SBUF evacuation (ACT), assembling g2^T [128 cols, 1024
     rows] directly — all overlapped with the other image's row pass.
  3. Column min-plus D2[j,i] = min_dd (dd^2 + g2T[j, i+dd]) over
     |dd| <= w on DVE in fp16 when w <= 44 (integers <= 2048 are fp16-
     exact; candidates in (2048, 4096] round by <= 1, a <= 0.05% error),
     f32 (STT pairs) otherwise. Odd shifts read a one-element-shifted
     copy to keep 4-byte alignment for the DVE 2x mode.
  4. Per-image global max via one small AllReduce (a dummy AllReduce at
     t=0 absorbs this runtime's ~55us first-collective barrier under the
     compute), then a short fp16 tail: masks compare unnormalized
     d = sqrt(D2) against 0.1*(max+1e-6), diff/abs/masked partial sums
     with fused accumulate; host sums the 8 partial pairs.
"""
import os
import sys

import numpy as np

for _p in ("/opt/trn_rl_repo", "/root/.axon_site/_ro/trn_rl_repo"):
    if os.path.isdir(_p) and _p not in sys.path:
        sys.path.append(_p)

import concourse.bacc as bacc
import concourse.tile as tile
from concourse import mybir
from concourse.bass_utils import run_bass_kernel_spmd

F32 = mybir.dt.float32
FP16 = mybir.dt.float16
I32 = mybir.dt.int32
AF = mybir.ActivationFunctionType
ALU = mybir.AluOpType
AX = mybir.AxisListType

H = 1024          # image height/width
P = 128           # partitions / rows per block / cols per core block
NB = 8            # row blocks per strip (H / P)
NCORES = 8
BIG = 1.0e4
INF = 1.0e9       # f32 sentinel
HINF = 60000.0    # fp16 sentinel (fp16 max normal is 65504)
FP16_WMAX = 44    # fp16 col pass iff w <= 44 (g^2, dd^2 <= 1936 exact)

_BUCKETS = (8, 10, 12, 14, 16, 18, 20, 22, 24, 26, 28, 32, 36, 40, 44,
            48, 56, 64, 80, 96, 128, 160, 192, 256, 320)


def _col_pass(tc, m, w, gTp, gB, persist, work):
    """Windowed min-plus; returns acc tile [P, H] (fp16 or f32).

    acc[j, i] = min_{|dd| <= w} (dd^2 + gTp[j, w + i + dd]); gTp is
    INF-padded by w on both sides. Entirely on DVE (tensor ops are
    rejected on Pool in this compiler build).
    """
    nc = tc.nc
    use16 = gB is not None

    if use16:
        def shifted(off):  # AP of width H at element offset `off` of gTp
            if off % 2 == 0:
                return gTp[:, off:off + H]
            return gB[:, off - 1:off - 1 + H]
    else:
        def shifted(off):
            return gTp[:, off:off + H]

    acc = persist.tile([P, H], FP16 if use16 else F32, tag=f"acc{m}")
    # Plain TT gets the DVE 2x mode for 16-bit and single-src TS gets 4x,
    # while the fused STT has no fast uop — so for fp16 a 3-op pairwise
    # form beats 2 STTs per dd. dd=1 folds the d=0 term.
    if use16:
        for dd in range(1, w + 1):
            tmp = work.tile([P, H], FP16, tag=f"pm{m}_{dd % 3}")
            nc.vector.tensor_tensor(tmp[:], shifted(w + dd), shifted(w - dd),
                                    ALU.min)
            nc.vector.tensor_scalar_add(tmp[:], tmp[:], float(dd * dd))
            nc.vector.tensor_tensor(
                acc[:], shifted(w) if dd == 1 else acc[:], tmp[:], ALU.min)
    else:
        for dd in range(1, w + 1):
            c = float(dd * dd)
            nc.vector.scalar_tensor_tensor(
                acc[:], shifted(w + dd), c,
                shifted(w) if dd == 1 else acc[:], ALU.add, ALU.min)
            nc.vector.scalar_tensor_tensor(
                acc[:], shifted(w - dd), c, acc[:], ALU.add, ALU.min)
    return acc


def _body(tc, w_gt, w_pred, srcs, partials, kind1='AllGather'):
    nc = tc.nc
    rg = [list(range(NCORES))]
    ws = (w_gt, w_pred)
    use16s = tuple(w <= FP16_WMAX for w in ws)

    with tc.tile_pool(name="const", bufs=1) as const, \
         tc.tile_pool(name="work", bufs=2) as work, \
         tc.tile_pool(name="persist", bufs=1) as persist, \
         tc.tile_pool(name="tail", bufs=1) as tail, \
         tc.tile_pool(name="ps", bufs=1, space="PSUM") as ps, \
         tc.tile_pool(name="dram", bufs=1, space="DRAM") as dram:

        # ---- input DMA ----
        # Both images ride one concatenated strip tensor, split across the
        # two HWDGE queues (SP and ACT) — DMA trigger instructions cost
        # ~0.7us each on the issuing queue.
        wds = [NB * (P + 2 * w) for w in ws]
        WD = sum(wds)
        st = persist.tile([P, WD], FP16, tag="st")
        for q in range(4):
            eng = nc.sync if q % 2 == 0 else nc.scalar
            eng.dma_start(st[q * 32:(q + 1) * 32, :],
                          srcs[q * 32:(q + 1) * 32, :])

        # ---- collective plumbing ----
        # Observed CC-stream behavior on this runtime: an automatic
        # barrier runs ~21.3us -> ~56-70us whether or not a collective has
        # been triggered yet; the first collective starts at
        # max(barrier_end, trigger) + ~11.5us and runs ~10us when its
        # trigger predates the barrier end but ~21us when triggered after
        # it; a collective_compute instruction blocks its issuing engine
        # queue until the collective completes. So: per-image AllReduces,
        # the first triggered right after the (shorter) gt column pass
        # ~48us — early enough for the fast path — doing useful work
        # instead of a dummy warm-up; the second queues behind it.
        ar_ins = [dram.tile([1, 1], F32, name=f"ari{m}", tag=f"ari{m}")
                  for m in range(2)]
        # collective #0 is an AllGather (fastest as the stream's first op:
        # ~4.5-9us vs ~10.5 for AllReduce) + a local 8-max; #1 is an
        # AllReduce (~12-13us vs ~16.3 for a second AllGather — any second
        # collective issued within ~10us of the first runs a ~2x slow
        # recovery mode, and AllReduce's is cheaper).
        ar_outs = [nc.dram_tensor("ar_out_sh0", [1, NCORES], F32,
                                  addr_space="Shared"),
                   nc.dram_tensor(
                       "ar_out_sh1",
                       [1, 1 if kind1 == "AllReduce" else NCORES], F32,
                       addr_space="Shared")]

        # ---- constants (DVE is idle while the strips stream in) ----
        io = const.tile([P, P], I32)
        nc.gpsimd.iota(io[:], [[1, P]], base=0, channel_multiplier=-1)
        ident = const.tile([P, P], F32)
        nc.vector.tensor_scalar(ident[:], io[:], 0, None, ALU.is_equal)
        identh = const.tile([P, P], FP16)
        nc.scalar.copy(identh[:], ident[:])
        ones1 = const.tile([1, P], F32)
        nc.vector.memset(ones1[:], 1.0)
        onesh = const.tile([P, WD], FP16)
        nc.vector.memset(onesh[:], 1.0)

        # ================= phase 1: row pass =================
        # One op each over the whole concatenation: the scans chain across
        # block/image boundaries, but carried-in state reaches any central
        # column with value > that image's margin and never wins.
        # foreground -> HINF, background -> 0. Host pre-scales inputs by
        # 1e30 (saturating fp16) so `> 0` is the fg test for both images
        # and fp16 underflow cannot flip tiny positives.
        z = work.tile([P, WD], FP16, tag="z")
        nc.vector.tensor_scalar(z[:], st[:], 0.0, HINF, ALU.is_gt, ALU.mult)
        dl = work.tile([P, WD], FP16, tag="dl")
        nc.vector.tensor_tensor_scan(dl[:], onesh[:], z[:], INF,
                                     ALU.add, ALU.min)
        dr = work.tile([P, WD], FP16, tag="dr")
        nc.vector.tensor_tensor_scan(dr[:, ::-1], onesh[:], z[:, ::-1], INF,
                                     ALU.add, ALU.min)
        g = work.tile([P, WD], FP16, tag="g")
        nc.vector.tensor_tensor(g[:], dl[:], dr[:], ALU.min)

        # ============ phase 2: transpose + square into g2^T ============
        gTps = []
        gBs = []
        for m in range(2):
            w = ws[m]
            use16 = use16s[m]
            dt = FP16 if use16 else F32
            inf = HINF if use16 else INF
            gw = H + 2 * w
            gTp = persist.tile([P, gw], dt, tag=f"gtp{m}")
            nc.vector.memset(gTp[:, :w], inf)
            nc.vector.memset(gTp[:, w + H:], inf)
            W = P + 2 * w
            base = m * wds[0]
            for b in range(NB):
                pt = ps.tile([P, P], FP16, tag="pt", bufs=4)
                off = base + b * W + w
                nc.tensor.transpose(pt[:], g[:, off:off + P], identh[:])
                nc.scalar.activation(gTp[:, w + b * P:w + (b + 1) * P], pt[:],
                                     AF.Square)
            if use16:
                # odd shifts read a one-element-shifted copy so the AP
                # stays 4-byte-aligned for the DVE 2x fp16 mode
                gB = persist.tile([P, gw], FP16, tag=f"gb{m}")
                nc.scalar.copy(gB[:, :gw - 1], gTp[:, 1:])
                nc.vector.memset(gB[:, gw - 1:], inf)
            else:
                gB = None
            gTps.append(gTp)
            gBs.append(gB)

        # ====== phase 3: column min-plus + per-image max + AllReduce ======
        # DVE queue order: col0, mx0, mxr0, col1, mx1, mxr1 — the tiny
        # partition-max of image 0 rides between the passes so AllReduce#0
        # triggers at ~48us, well before the CC barrier ends.
        ys = []
        mxrs = []
        accs = []
        for m in range(2):
            acc = _col_pass(tc, m, ws[m], gTps[m], gBs[m], persist, work)
            accs.append(acc)
            mxp = tail.tile([P, 1], F32, tag=f"mxp{m}")
            nc.vector.reduce_max(mxp[:], acc[:], axis=AX.X)
            pmx = ps.tile([1, P], F32, tag="pmx")
            nc.tensor.transpose(pmx[:], mxp[:], ident[:])
            mxr = tail.tile([1, 1], F32, tag=f"mxr{m}")
            nc.vector.reduce_max(mxr[:], pmx[:], axis=AX.X)
            mxrs.append(mxr)
            nc.sync.dma_start(ar_ins[m][0:1, 0:1], mxr[:])
            if m == 0:
                nc.gpsimd.collective_compute(
                    "AllGather", ALU.bypass, replica_groups=rg,
                    ins=[ar_ins[m][:, :].opt()],
                    outs=[ar_outs[m][:, :].opt()])
            elif kind1 == "AllReduce":
                nc.gpsimd.collective_compute(
                    "AllReduce", ALU.max, replica_groups=rg,
                    ins=[ar_ins[m][:, :].opt()],
                    outs=[ar_outs[m][:, :].opt()])
            else:
                nc.gpsimd.collective_compute(
                    "AllGather", ALU.bypass, replica_groups=rg,
                    ins=[ar_ins[m][:, :].opt()],
                    outs=[ar_outs[m][:, :].opt()])
            # unnormalized distances, precomputed before AllReduce#1 ends
            y = persist.tile([P, H], FP16, tag=f"y{m}")
            nc.scalar.activation(y[:], acc[:], AF.Sqrt)
            ys.append(y)

        # ============ phase 4: per-image inv = 1/(sqrt(max)+1e-6) ============
        # Image 0's chain (and a0 below) runs while AllReduce#1 is in
        # flight; image 1's is the post-collective critical path. The DVE
        # bits are emitted after both column passes so the in-order DVE
        # queue never stalls on a collective result mid-compute.
        def inv_chain(m, guard=None):
            # Fetched on the ACT HWDGE queue: on the SP queue this trigger
            # (gated on the collective's completion) gets scheduled ahead
            # of collective #1's input staging and head-of-line blocks it.
            if m == 1 and kind1 == "AllReduce":
                gmx = tail.tile([1, 1], F32, tag="gmx1")
                nc.scalar.dma_start(gmx[:], ar_outs[1][0:1, 0:1])
            elif m == 1:
                gmx8b = tail.tile([1, NCORES], F32, tag="gmx8b")
                nc.scalar.dma_start(gmx8b[:], ar_outs[1][0:1, :])
                gmx = tail.tile([1, 1], F32, tag="gmx1")
                nc.vector.reduce_max(gmx[:], gmx8b[:], axis=AX.X)
            else:
                gmx8 = tail.tile([1, NCORES], F32, tag=f"gmx8{m}")
                nc.scalar.dma_start(gmx8[:], ar_outs[m][0:1, :])
                gmx = tail.tile([1, 1], F32, tag=f"gmx{m}")
                assert guard is not None
                # The scheduler also hoists DVE ops above earlier-created
                # ones; a collective-gated op hoisted above the image-1
                # column pass would head-of-line block the DVE queue (and
                # with it AllGather#1's trigger). op0=bypass both ignores
                # the `guard` scalar AND records a read of it, pinning
                # this op (and its dependents) behind the column pass.
                t4 = tail.tile([1, 4], F32, tag="t4")
                nc.vector.scalar_tensor_tensor(
                    t4[:], gmx8[:, 0:4], guard, gmx8[:, 4:8],
                    ALU.bypass, ALU.max)
                t2 = tail.tile([1, 2], F32, tag="t2")
                nc.vector.tensor_tensor(t2[:], t4[:, 0:2], t4[:, 2:4],
                                        ALU.max)
                nc.vector.tensor_tensor(gmx[:], t2[:, 0:1], t2[:, 1:2],
                                        ALU.max)
            msq = tail.tile([1, 1], F32, tag=f"msq{m}")
            nc.scalar.activation(msq[:], gmx[:], AF.Sqrt)
            s1 = tail.tile([1, 1], F32, tag=f"s1{m}")
            nc.vector.tensor_scalar_add(s1[:], msq[:], 1e-6)
            nc.vector.reciprocal(s1[:], s1[:])
            pb = ps.tile([P, 1], F32, tag="pb")
            nc.tensor.matmul(pb[:], ones1[:], s1[:])
            invb = tail.tile([P, 1], F32, tag=f"invb{m}")
            nc.scalar.copy(invb[:], pb[:])
            return invb

        # ================= phase 5: normalize + masked mean =================
        # Image 0's inv chain and a0 (on ACT; scale is a per-partition AP)
        # overlap AllReduce#1; image 1's chain and a1 (DVE) are the
        # post-collective critical path.
        invb0 = inv_chain(0, guard=mxrs[1][0:1, 0:1])
        a0 = tail.tile([P, H], FP16, tag="a0")
        nc.scalar.activation(a0[:], ys[0][:], AF.Copy, scale=invb0[:, 0:1])
        invb1 = inv_chain(1)
        a1 = tail.tile([P, H], FP16, tag="a1")
        nc.vector.tensor_scalar(a1[:], ys[1][:], invb1[:, 0:1], None,
                                ALU.mult)
        # (a0 < 0.1) | (a1 < 0.1)  ==  min(a0, a1) < 0.1; the masked-|diff|
        # sum and the mask count ride the ACT accumulators (|mk*df| =
        # mk*|df|), keeping only 5 ops on the post-collective DVE path.
        mk = tail.tile([P, H], FP16, tag="mk")
        nc.vector.tensor_tensor(mk[:], a0[:], a1[:], ALU.min)
        nc.vector.tensor_scalar(mk[:], mk[:], 0.1, None, ALU.is_lt)
        df = tail.tile([P, H], FP16, tag="df")
        nc.vector.tensor_tensor(df[:], a0[:], a1[:], ALU.subtract)
        nc.vector.tensor_tensor(df[:], df[:], mk[:], ALU.mult)
        # Per-partition partial sums go out as [128, 2]; the host finishes
        # the 256-element reduction (frees the PE matmul + PSUM-evacuate
        # hops from the post-collective critical path).
        s12 = tail.tile([P, 2], F32, tag="s12")
        cnt = tail.tile([P, H], FP16, tag="cnt")
        nc.scalar.activation(cnt[:], mk[:], AF.Copy, accum_out=s12[:, 1:2])
        da = tail.tile([P, H], FP16, tag="da")
        nc.scalar.activation(da[:], df[:], AF.Abs, accum_out=s12[:, 0:1])
        nc.sync.dma_start(partials[:, :], s12[:])


def _build(w_gt, w_pred, kind1="AllGather"):
    nc = bacc.Bacc("TRN2", target_bir_lowering=False, debug=False,
                   num_devices=NCORES)
    wd = NB * (P + 2 * w_gt) + NB * (P + 2 * w_pred)
    st = nc.dram_tensor("st", [P, wd], FP16, kind="ExternalInput")
    partials = nc.dram_tensor("partials", [P, 2], F32,
                               kind="ExternalOutput")
    with tile.TileContext(nc) as tc:
        _body(tc, w_gt, w_pred, st, partials, kind1)
    nc.compile()
    return nc


_PROGRAMS = {}


def _program(*key):
    if key not in _PROGRAMS:
        _PROGRAMS[key] = _build(*key)
    return _PROGRAMS[key]


def _row_gmax(fg):
    """Max over pixels of the in-row distance to the nearest background
    pixel (clamped to BIG). This equals the exact column-pass window bound."""
    idx = np.arange(fg.shape[1], dtype=np.float64)
    zero = ~fg
    left = np.maximum.accumulate(np.where(zero, idx, -np.inf), axis=1)
    right = np.minimum.accumulate(np.where(zero, idx, np.inf)[:, ::-1],
                                  axis=1)[:, ::-1]
    g = np.minimum(np.minimum(idx - left, right - idx), BIG)
    return float(g.max())


def _bucket(gmax):
    need = min(int(np.ceil(gmax)), H - 1)
    for b in _BUCKETS:
        if b >= need:
            return b
    raise NotImplementedError(
        f"row gmax {gmax} exceeds the supported strip margin {_BUCKETS[-1]}")


def _strips(img, w):
    """Per-core fp16 strips [128, 8*(128+2w)]: strip[c][p, b*(128+2w)+q] =
    scaled img[128*b + p, 128*c - w + q], fg-padded outside the image."""
    x = np.asarray(img, np.float32) * 1e30
    pad = np.full((H, w), np.float32(1e30))
    xp = np.concatenate([pad, x, pad], axis=1)
    W = P + 2 * w
    out = []
    for c in range(NCORES):
        b = xp[:, c * P:c * P + W].astype(np.float16)
        out.append(np.ascontiguousarray(
            b.reshape(NB, P, W).transpose(1, 0, 2).reshape(P, NB * W)))
    return out


def _run(pred, gt, trace=False):
    pred = np.ascontiguousarray(np.asarray(pred), dtype=np.float32)
    gt = np.ascontiguousarray(np.asarray(gt), dtype=np.float32)
    assert pred.shape == (H, H) and gt.shape == (H, H)
    w_gt = _bucket(_row_gmax(gt != 0))
    w_pred = _bucket(_row_gmax(pred > 0))
    nc = _program(w_gt, w_pred)
    sg = _strips(gt, w_gt)
    sp = _strips(pred, w_pred)
    in_maps = [{"st": np.ascontiguousarray(
        np.concatenate([sg[c], sp[c]], axis=1))} for c in range(NCORES)]
    res = run_bass_kernel_spmd(nc, in_maps, list(range(NCORES)), trace=trace)
    tot = np.zeros(2, np.float64)
    for r in res.results:
        tot += np.asarray(r["partials"], np.float64).reshape(P, 2).sum(0)
    loss = np.float32(tot[0] / max(tot[1], 1.0))
    return loss, res


def kernel(pred, gt):
    loss, _ = _run(pred, gt)
    return loss


# revision 49
# speedup vs baseline: 1.0216x; 1.0165x over previous
"""BoundaryLoss Trainium2 kernel (8 NeuronCores, SPMD, strip-replicated).

Layout: core c owns output column block [128c, 128c+128). The host hands
each core a strip of every input row covering its block plus a margin of
w columns on each side (w = bucketed max in-row nearest-background
distance, measured exactly on the host as in the previous revision).
Row-local EDT distances never exceed w at the central columns, so each
core can run the full row pass locally — no AllToAll at all, which in the
prior revision serialized ~70us of collective latency ahead of the column
pass.

Pipeline (per core):
  1. Row pass on [128, 8*W] fp16 strips (W = 128+2w; partition p, block b
     holds image row 128b+p). One forward + one reverse
     tensor_tensor_scan per image; the scan chains across block
     boundaries, but any carried-in state reaches a central column with
     value > w and so never wins (margin absorbs it).
  2. PE-transpose the central 128 columns of each block (g, fp16), square
     on the PSUM->SBUF evacuation (ACT), assembling g2^T [128 cols, 1024
     rows] directly — all overlapped with the other image's row pass.
  3. Column min-plus D2[j,i] = min_dd (dd^2 + g2T[j, i+dd]) over
     |dd| <= w on DVE in fp16 when w <= 44 (integers <= 2048 are fp16-
     exact; candidates in (2048, 4096] round by <= 1, a <= 0.05% error),
     f32 (STT pairs) otherwise. Odd shifts read a one-element-shifted
     copy to keep 4-byte alignment for the DVE 2x mode.
  4. Per-image global max via one small AllReduce (a dummy AllReduce at
     t=0 absorbs this runtime's ~55us first-collective barrier under the
     compute), then a short fp16 tail: masks compare unnormalized
     d = sqrt(D2) against 0.1*(max+1e-6), diff/abs/masked partial sums
     with fused accumulate; host sums the 8 partial pairs.
"""
import os
import sys

import numpy as np

for _p in ("/opt/trn_rl_repo", "/root/.axon_site/_ro/trn_rl_repo"):
    if os.path.isdir(_p) and _p not in sys.path:
        sys.path.append(_p)

import concourse.bacc as bacc
import concourse.tile as tile
from concourse import mybir
from concourse.bass_utils import run_bass_kernel_spmd

F32 = mybir.dt.float32
FP16 = mybir.dt.float16
I32 = mybir.dt.int32
AF = mybir.ActivationFunctionType
ALU = mybir.AluOpType
AX = mybir.AxisListType

H = 1024          # image height/width
P = 128           # partitions / rows per block / cols per core block
NB = 8            # row blocks per strip (H / P)
NCORES = 8
BIG = 1.0e4
INF = 1.0e9       # f32 sentinel
HINF = 60000.0    # fp16 sentinel (fp16 max normal is 65504)
FP16_WMAX = 44    # fp16 col pass iff w <= 44 (g^2, dd^2 <= 1936 exact)

_BUCKETS = (8, 10, 12, 14, 16, 18, 20, 22, 24, 26, 28, 32, 36, 40, 44,
            48, 56, 64, 80, 96, 128, 160, 192, 256, 320)


def _col_pass(tc, m, w, gTp, gB, persist, work):
    """Windowed min-plus; returns acc tile [P, H] (fp16 or f32).

    acc[j, i] = min_{|dd| <= w} (dd^2 + gTp[j, w + i + dd]); gTp is
    INF-padded by w on both sides. Entirely on DVE (tensor ops are
    rejected on Pool in this compiler build).
    """
    nc = tc.nc
    use16 = gB is not None

    if use16:
        def shifted(off):  # AP of width H at element offset `off` of gTp
            if off % 2 == 0:
                return gTp[:, off:off + H]
            return gB[:, off - 1:off - 1 + H]
    else:
        def shifted(off):
            return gTp[:, off:off + H]

    acc = persist.tile([P, H], FP16 if use16 else F32, tag=f"acc{m}")
    # Plain TT gets the DVE 2x mode for 16-bit and single-src TS gets 4x,
    # while the fused STT has no fast uop — so for fp16 a 3-op pairwise
    # form beats 2 STTs per dd. dd=1 folds the d=0 term.
    if use16:
        for dd in range(1, w + 1):
            tmp = work.tile([P, H], FP16, tag=f"pm{m}_{dd % 3}")
            nc.vector.tensor_tensor(tmp[:], shifted(w + dd), shifted(w - dd),
                                    ALU.min)
            nc.vector.tensor_scalar_add(tmp[:], tmp[:], float(dd * dd))
            nc.vector.tensor_tensor(
                acc[:], shifted(w) if dd == 1 else acc[:], tmp[:], ALU.min)
    else:
        for dd in range(1, w + 1):
            c = float(dd * dd)
            nc.vector.scalar_tensor_tensor(
                acc[:], shifted(w + dd), c,
                shifted(w) if dd == 1 else acc[:], ALU.add, ALU.min)
            nc.vector.scalar_tensor_tensor(
                acc[:], shifted(w - dd), c, acc[:], ALU.add, ALU.min)
    return acc


def _body(tc, w_gt, w_pred, srcs, partials, kind1='AllGather'):
    nc = tc.nc
    rg = [list(range(NCORES))]
    ws = (w_gt, w_pred)
    use16s = tuple(w <= FP16_WMAX for w in ws)

    with tc.tile_pool(name="const", bufs=1) as const, \
         tc.tile_pool(name="work", bufs=2) as work, \
         tc.tile_pool(name="persist", bufs=1) as persist, \
         tc.tile_pool(name="tail", bufs=1) as tail, \
         tc.tile_pool(name="ps", bufs=1, space="PSUM") as ps, \
         tc.tile_pool(name="dram", bufs=1, space="DRAM") as dram:

        # ---- input DMA ----
        # Both images ride one concatenated strip tensor, split across the
        # two HWDGE queues (SP and ACT) — DMA trigger instructions cost
        # ~0.7us each on the issuing queue.
        wds = [NB * (P + 2 * w) for w in ws]
        WD = sum(wds)
        st = persist.tile([P, WD], FP16, tag="st")
        for q in range(4):
            eng = nc.sync if q % 2 == 0 else nc.scalar
            eng.dma_start(st[q * 32:(q + 1) * 32, :],
                          srcs[q * 32:(q + 1) * 32, :])

        # ---- collective plumbing ----
        # Observed CC-stream behavior on this runtime: an automatic
        # barrier runs ~21.3us -> ~56-70us whether or not a collective has
        # been triggered yet; the first collective starts at
        # max(barrier_end, trigger) + ~11.5us and runs ~10us when its
        # trigger predates the barrier end but ~21us when triggered after
        # it; a collective_compute instruction blocks its issuing engine
        # queue until the collective completes. So: per-image AllReduces,
        # the first triggered right after the (shorter) gt column pass
        # ~48us — early enough for the fast path — doing useful work
        # instead of a dummy warm-up; the second queues behind it.
        ar_ins = [dram.tile([1, 1], F32, name=f"ari{m}", tag=f"ari{m}")
                  for m in range(2)]
        # collective #0 is an AllGather (fastest as the stream's first op:
        # ~4.5-9us vs ~10.5 for AllReduce) + a local 8-max; #1 is an
        # AllReduce (~12-13us vs ~16.3 for a second AllGather — any second
        # collective issued within ~10us of the first runs a ~2x slow
        # recovery mode, and AllReduce's is cheaper).
        ar_outs = [nc.dram_tensor("ar_out_sh0", [1, NCORES], F32,
                                  addr_space="Shared"),
                   nc.dram_tensor(
                       "ar_out_sh1",
                       [1, 1 if kind1 == "AllReduce" else NCORES], F32,
                       addr_space="Shared")]

        # ---- constants (DVE is idle while the strips stream in) ----
        io = const.tile([P, P], I32)
        nc.gpsimd.iota(io[:], [[1, P]], base=0, channel_multiplier=-1)
        ident = const.tile([P, P], F32)
        nc.vector.tensor_scalar(ident[:], io[:], 0, None, ALU.is_equal)
        identh = const.tile([P, P], FP16)
        nc.scalar.copy(identh[:], ident[:])
        ones1 = const.tile([1, P], F32)
        nc.vector.memset(ones1[:], 1.0)
        onesh = const.tile([P, WD], FP16)
        nc.vector.memset(onesh[:], 1.0)

        # ============ phases 1+2: row pass + transpose, interleaved ============
        # Per-image row pass on slices of the concatenated strip; image
        # 0's PE transposes + squares run while image 1's scans occupy
        # DVE, so the first column pass starts without a stall. The scans
        # chain across block boundaries, but carried-in state reaches any
        # central column with value > the margin and never wins.
        # foreground -> HINF, background -> 0. Host pre-scales inputs by
        # 1e30 (saturating fp16) so `> 0` is the fg test for both images
        # and fp16 underflow cannot flip tiny positives.
        gTps = []
        gBs = []
        for m in range(2):
            w = ws[m]
            wd = wds[m]
            lo = m * wds[0]
            z = work.tile([P, wd], FP16, tag=f"z{m}")
            nc.vector.tensor_scalar(z[:], st[:, lo:lo + wd], 0.0, HINF,
                                    ALU.is_gt, ALU.mult)
            dl = work.tile([P, wd], FP16, tag=f"dl{m}")
            nc.vector.tensor_tensor_scan(dl[:], onesh[:, :wd], z[:], INF,
                                         ALU.add, ALU.min)
            dr = work.tile([P, wd], FP16, tag=f"dr{m}")
            nc.vector.tensor_tensor_scan(dr[:, ::-1], onesh[:, :wd],
                                         z[:, ::-1], INF, ALU.add, ALU.min)
            g = work.tile([P, wd], FP16, tag=f"g{m}")
            nc.vector.tensor_tensor(g[:], dl[:], dr[:], ALU.min)

            use16 = use16s[m]
            dt = FP16 if use16 else F32
            inf = HINF if use16 else INF
            gw = H + 2 * w
            gTp = persist.tile([P, gw], dt, tag=f"gtp{m}")
            nc.vector.memset(gTp[:, :w], inf)
            nc.vector.memset(gTp[:, w + H:], inf)
            W = P + 2 * w
            for b in range(NB):
                pt = ps.tile([P, P], FP16, tag="pt", bufs=4)
                off = b * W + w
                nc.tensor.transpose(pt[:], g[:, off:off + P], identh[:])
                nc.scalar.activation(gTp[:, w + b * P:w + (b + 1) * P], pt[:],
                                     AF.Square)
            if use16:
                # odd shifts read a one-element-shifted copy so the AP
                # stays 4-byte-aligned for the DVE 2x fp16 mode
                gB = persist.tile([P, gw], FP16, tag=f"gb{m}")
                nc.scalar.copy(gB[:, :gw - 1], gTp[:, 1:])
                nc.vector.memset(gB[:, gw - 1:], inf)
            else:
                gB = None
            gTps.append(gTp)
            gBs.append(gB)

        # ====== phase 3: column min-plus + per-image max + AllReduce ======
        # DVE queue order: col0, mx0, mxr0, col1, mx1, mxr1 — the tiny
        # partition-max of image 0 rides between the passes so AllReduce#0
        # triggers at ~48us, well before the CC barrier ends.
        ys = []
        mxrs = []
        accs = []
        for m in range(2):
            acc = _col_pass(tc, m, ws[m], gTps[m], gBs[m], persist, work)
            accs.append(acc)
            mxp = tail.tile([P, 1], F32, tag=f"mxp{m}")
            nc.vector.reduce_max(mxp[:], acc[:], axis=AX.X)
            pmx = ps.tile([1, P], F32, tag="pmx")
            nc.tensor.transpose(pmx[:], mxp[:], ident[:])
            mxr = tail.tile([1, 1], F32, tag=f"mxr{m}")
            nc.vector.reduce_max(mxr[:], pmx[:], axis=AX.X)
            mxrs.append(mxr)
            nc.sync.dma_start(ar_ins[m][0:1, 0:1], mxr[:])
            if m == 0:
                nc.gpsimd.collective_compute(
                    "AllGather", ALU.bypass, replica_groups=rg,
                    ins=[ar_ins[m][:, :].opt()],
                    outs=[ar_outs[m][:, :].opt()])
            elif kind1 == "AllReduce":
                nc.gpsimd.collective_compute(
                    "AllReduce", ALU.max, replica_groups=rg,
                    ins=[ar_ins[m][:, :].opt()],
                    outs=[ar_outs[m][:, :].opt()])
            else:
                nc.gpsimd.collective_compute(
                    "AllGather", ALU.bypass, replica_groups=rg,
                    ins=[ar_ins[m][:, :].opt()],
                    outs=[ar_outs[m][:, :].opt()])
            # unnormalized distances, precomputed before AllReduce#1 ends
            y = persist.tile([P, H], FP16, tag=f"y{m}")
            nc.scalar.activation(y[:], acc[:], AF.Sqrt)
            ys.append(y)

        # ============ phase 4: per-image inv = 1/(sqrt(max)+1e-6) ============
        # Image 0's chain (and a0 below) runs while AllReduce#1 is in
        # flight; image 1's is the post-collective critical path. The DVE
        # bits are emitted after both column passes so the in-order DVE
        # queue never stalls on a collective result mid-compute.
        def inv_chain(m, guard=None):
            # Fetched on the ACT HWDGE queue: on the SP queue this trigger
            # (gated on the collective's completion) gets scheduled ahead
            # of collective #1's input staging and head-of-line blocks it.
            if m == 1 and kind1 == "AllReduce":
                gmx = tail.tile([1, 1], F32, tag="gmx1")
                nc.scalar.dma_start(gmx[:], ar_outs[1][0:1, 0:1])
            elif m == 1:
                gmx8b = tail.tile([1, NCORES], F32, tag="gmx8b")
                nc.scalar.dma_start(gmx8b[:], ar_outs[1][0:1, :])
                gmx = tail.tile([1, 1], F32, tag="gmx1")
                nc.vector.reduce_max(gmx[:], gmx8b[:], axis=AX.X)
            else:
                gmx8 = tail.tile([1, NCORES], F32, tag=f"gmx8{m}")
                nc.scalar.dma_start(gmx8[:], ar_outs[m][0:1, :])
                gmx = tail.tile([1, 1], F32, tag=f"gmx{m}")
                assert guard is not None
                # The scheduler also hoists DVE ops above earlier-created
                # ones; a collective-gated op hoisted above the image-1
                # column pass would head-of-line block the DVE queue (and
                # with it AllGather#1's trigger). op0=bypass both ignores
                # the `guard` scalar AND records a read of it, pinning
                # this op (and its dependents) behind the column pass.
                t4 = tail.tile([1, 4], F32, tag="t4")
                nc.vector.scalar_tensor_tensor(
                    t4[:], gmx8[:, 0:4], guard, gmx8[:, 4:8],
                    ALU.bypass, ALU.max)
                t2 = tail.tile([1, 2], F32, tag="t2")
                nc.vector.tensor_tensor(t2[:], t4[:, 0:2], t4[:, 2:4],
                                        ALU.max)
                nc.vector.tensor_tensor(gmx[:], t2[:, 0:1], t2[:, 1:2],
                                        ALU.max)
            msq = tail.tile([1, 1], F32, tag=f"msq{m}")
            nc.scalar.activation(msq[:], gmx[:], AF.Sqrt)
            s1 = tail.tile([1, 1], F32, tag=f"s1{m}")
            nc.vector.tensor_scalar_add(s1[:], msq[:], 1e-6)
            nc.vector.reciprocal(s1[:], s1[:])
            pb = ps.tile([P, 1], F32, tag="pb")
            nc.tensor.matmul(pb[:], ones1[:], s1[:])
            invb = tail.tile([P, 1], F32, tag=f"invb{m}")
            nc.scalar.copy(invb[:], pb[:])
            return invb

        # ================= phase 5: normalize + masked mean =================
        # Image 0's inv chain and a0 (on ACT; scale is a per-partition AP)
        # overlap AllReduce#1; image 1's chain and a1 (DVE) are the
        # post-collective critical path.
        invb0 = inv_chain(0, guard=mxrs[1][0:1, 0:1])
        a0 = tail.tile([P, H], FP16, tag="a0")
        nc.scalar.activation(a0[:], ys[0][:], AF.Copy, scale=invb0[:, 0:1])
        invb1 = inv_chain(1)
        a1 = tail.tile([P, H], FP16, tag="a1")
        nc.vector.tensor_scalar(a1[:], ys[1][:], invb1[:, 0:1], None,
                                ALU.mult)
        # (a0 < 0.1) | (a1 < 0.1)  ==  min(a0, a1) < 0.1; the masked-|diff|
        # sum and the mask count ride the ACT accumulators (|mk*df| =
        # mk*|df|), keeping only 5 ops on the post-collective DVE path.
        mk = tail.tile([P, H], FP16, tag="mk")
        nc.vector.tensor_tensor(mk[:], a0[:], a1[:], ALU.min)
        nc.vector.tensor_scalar(mk[:], mk[:], 0.1, None, ALU.is_lt)
        df = tail.tile([P, H], FP16, tag="df")
        nc.vector.tensor_tensor(df[:], a0[:], a1[:], ALU.subtract)
        nc.vector.tensor_tensor(df[:], df[:], mk[:], ALU.mult)
        # Per-partition partial sums go out as [128, 2]; the host finishes
        # the 256-element reduction (frees the PE matmul + PSUM-evacuate
        # hops from the post-collective critical path).
        s12 = tail.tile([P, 2], F32, tag="s12")
        cnt = tail.tile([P, H], FP16, tag="cnt")
        nc.scalar.activation(cnt[:], mk[:], AF.Copy, accum_out=s12[:, 1:2])
        da = tail.tile([P, H], FP16, tag="da")
        nc.scalar.activation(da[:], df[:], AF.Abs, accum_out=s12[:, 0:1])
        nc.sync.dma_start(partials[:, :], s12[:])


def _build(w_gt, w_pred, kind1="AllGather"):
    nc = bacc.Bacc("TRN2", target_bir_lowering=False, debug=False,
                   num_devices=NCORES)
    wd = NB * (P + 2 * w_gt) + NB * (P + 2 * w_pred)
    st = nc.dram_tensor("st", [P, wd], FP16, kind="ExternalInput")
    partials = nc.dram_tensor("partials", [P, 2], F32,
                               kind="ExternalOutput")
    with tile.TileContext(nc) as tc:
        _body(tc, w_gt, w_pred, st, partials, kind1)
    nc.compile()
    return nc


_PROGRAMS = {}


def _program(*key):
    if key not in _PROGRAMS:
        _PROGRAMS[key] = _build(*key)
    return _PROGRAMS[key]


def _row_gmax(fg):
    """Max over pixels of the in-row distance to the nearest background
    pixel (clamped to BIG). This equals the exact column-pass window bound."""
    idx = np.arange(fg.shape[1], dtype=np.float64)
    zero = ~fg
    left = np.maximum.accumulate(np.where(zero, idx, -np.inf), axis=1)
    right = np.minimum.accumulate(np.where(zero, idx, np.inf)[:, ::-1],
                                  axis=1)[:, ::-1]
    g = np.minimum(np.minimum(idx - left, right - idx), BIG)
    return float(g.max())


def _bucket(gmax):
    need = min(int(np.ceil(gmax)), H - 1)
    for b in _BUCKETS:
        if b >= need:
            return b
    raise NotImplementedError(
        f"row gmax {gmax} exceeds the supported strip margin {_BUCKETS[-1]}")


def _strips(img, w):
    """Per-core fp16 strips [128, 8*(128+2w)]: strip[c][p, b*(128+2w)+q] =
    scaled img[128*b + p, 128*c - w + q], fg-padded outside the image."""
    x = np.asarray(img, np.float32) * 1e30
    pad = np.full((H, w), np.float32(1e30))
    xp = np.concatenate([pad, x, pad], axis=1)
    W = P + 2 * w
    out = []
    for c in range(NCORES):
        b = xp[:, c * P:c * P + W].astype(np.float16)
        out.append(np.ascontiguousarray(
            b.reshape(NB, P, W).transpose(1, 0, 2).reshape(P, NB * W)))
    return out


def _run(pred, gt, trace=False):
    pred = np.ascontiguousarray(np.asarray(pred), dtype=np.float32)
    gt = np.ascontiguousarray(np.asarray(gt), dtype=np.float32)
    assert pred.shape == (H, H) and gt.shape == (H, H)
    w_gt = _bucket(_row_gmax(gt != 0))
    w_pred = _bucket(_row_gmax(pred > 0))
    nc = _program(w_gt, w_pred)
    sg = _strips(gt, w_gt)
    sp = _strips(pred, w_pred)
    in_maps = [{"st": np.ascontiguousarray(
        np.concatenate([sg[c], sp[c]], axis=1))} for c in range(NCORES)]
    res = run_bass_kernel_spmd(nc, in_maps, list(range(NCORES)), trace=trace)
    tot = np.zeros(2, np.float64)
    for r in res.results:
        tot += np.asarray(r["partials"], np.float64).reshape(P, 2).sum(0)
    loss = np.float32(tot[0] / max(tot[1], 1.0))
    return loss, res


def kernel(pred, gt):
    loss, _ = _run(pred, gt)
    return loss


# revision 52
# speedup vs baseline: 1.0836x; 1.0607x over previous
"""BoundaryLoss Trainium2 kernel (8 NeuronCores, SPMD, strip-replicated).

Layout: core c owns output column block [128c, 128c+128). The host hands
each core a strip of every input row covering its block plus a margin of
w columns on each side (w = bucketed max in-row nearest-background
distance, measured exactly on the host as in the previous revision).
Row-local EDT distances never exceed w at the central columns, so each
core can run the full row pass locally — no AllToAll at all, which in the
prior revision serialized ~70us of collective latency ahead of the column
pass.

Pipeline (per core):
  1. Row pass on fp16 strips (both images concatenated into one [128,
     8*W0 + 8*W1] input; W = 128+2w; partition p, block b holds image row
     128b+p). One forward + one reverse tensor_tensor_scan per image; the
     scan chains across block boundaries, but any carried-in state
     reaches a central column with value > w and so never wins.
  2. PE-transpose the central 128 columns of each block (g, fp16), square
     on the PSUM->SBUF evacuation (ACT), assembling g2^T [128 cols, 1024
     rows] directly — image 0's transposes overlap image 1's scans.
  3. Column min-plus D2[j,i] = min_dd (dd^2 + g2T[j, i+dd]) over
     |dd| <= w on DVE in fp16 when w <= 44 (integers <= 2048 are fp16-
     exact; candidates in (2048, 4096] round by <= 1, a <= 0.05% error),
     f32 (STT pairs) otherwise. Odd shifts read a one-element-shifted
     copy to keep 4-byte alignment for the DVE 2x mode.
  4. Per-image global max via two tiny AllGathers: #0 (gt) triggers right
     after the first column pass (~48us), inside the CC stream's
     automatic start barrier — absorbing the ~11.5us first-collective
     launch — and #1 (pred) queues right behind it. A short fp16 tail
     (mask = min(a0,a1) < 0.1, |a0-a1|, ACT-accumulated partial sums)
     lands per-partition partials [128, 2] that the host finishes.

Timing notes for this runtime (medians; all components vary run-to-run):
  NEFF preamble ~10.4us; CC start barrier ~21.5us -> +25..50us; first
  collective = max(barrier_end, trigger) + 11.5us, ~8-10us duration
  (AllGather as first op beats AllReduce ~10.5, and a late trigger runs
  ~2x slower); a second collective within ~10us of the first runs a slow
  ~16us mode (AllGather) / ~19us (AllReduce). The scheduler REORDERS ops
  within an engine queue: any op gated on a collective result can
  head-of-line block later ops (see the bypass-guard in inv_chain and
  the ACT-queue output fetches).
"""
import os
import sys

import numpy as np

for _p in ("/opt/trn_rl_repo", "/root/.axon_site/_ro/trn_rl_repo"):
    if os.path.isdir(_p) and _p not in sys.path:
        sys.path.append(_p)

import concourse.bacc as bacc
import concourse.tile as tile
from concourse import mybir
from concourse.bass_utils import run_bass_kernel_spmd

F32 = mybir.dt.float32
FP16 = mybir.dt.float16
I32 = mybir.dt.int32
AF = mybir.ActivationFunctionType
ALU = mybir.AluOpType
AX = mybir.AxisListType

H = 1024          # image height/width
P = 128           # partitions / rows per block / cols per core block
NB = 8            # row blocks per strip (H / P)
NCORES = 8
BIG = 1.0e4
INF = 1.0e9       # f32 sentinel
HINF = 60000.0    # fp16 sentinel (fp16 max normal is 65504)
FP16_WMAX = 44    # fp16 col pass iff w <= 44 (g^2, dd^2 <= 1936 exact)

_BUCKETS = (8, 10, 12, 14, 16, 18, 20, 22, 24, 26, 28, 32, 36, 40, 44,
            48, 56, 64, 80, 96, 128, 160, 192, 256, 320)


def _col_pass(tc, m, w, gTp, gB, persist, work):
    """Windowed min-plus; returns acc tile [P, H] (fp16 or f32).

    acc[j, i] = min_{|dd| <= w} (dd^2 + gTp[j, w + i + dd]); gTp is
    INF-padded by w on both sides. Entirely on DVE (tensor ops are
    rejected on Pool in this compiler build).
    """
    nc = tc.nc
    use16 = gB is not None

    if use16:
        def shifted(off):  # AP of width H at element offset `off` of gTp
            if off % 2 == 0:
                return gTp[:, off:off + H]
            return gB[:, off - 1:off - 1 + H]
    else:
        def shifted(off):
            return gTp[:, off:off + H]

    acc = persist.tile([P, H], FP16 if use16 else F32, tag=f"acc{m}")
    # Plain TT gets the DVE 2x mode for 16-bit and single-src TS gets 4x,
    # while the fused STT has no fast uop — so for fp16 a 3-op pairwise
    # form beats 2 STTs per dd. dd=1 folds the d=0 term.
    if use16:
        for dd in range(1, w + 1):
            tmp = work.tile([P, H], FP16, tag=f"pm{m}_{dd % 3}")
            nc.vector.tensor_tensor(tmp[:], shifted(w + dd), shifted(w - dd),
                                    ALU.min)
            nc.vector.tensor_scalar_add(tmp[:], tmp[:], float(dd * dd))
            nc.vector.tensor_tensor(
                acc[:], shifted(w) if dd == 1 else acc[:], tmp[:], ALU.min)
    else:
        for dd in range(1, w + 1):
            c = float(dd * dd)
            nc.vector.scalar_tensor_tensor(
                acc[:], shifted(w + dd), c,
                shifted(w) if dd == 1 else acc[:], ALU.add, ALU.min)
            nc.vector.scalar_tensor_tensor(
                acc[:], shifted(w - dd), c, acc[:], ALU.add, ALU.min)
    return acc


def _body(tc, w_gt, w_pred, srcs, partials, kind1='AllGather'):
    nc = tc.nc
    rg = [list(range(NCORES))]
    ws = (w_gt, w_pred)
    use16s = tuple(w <= FP16_WMAX for w in ws)

    with tc.tile_pool(name="const", bufs=1) as const, \
         tc.tile_pool(name="work", bufs=2) as work, \
         tc.tile_pool(name="persist", bufs=1) as persist, \
         tc.tile_pool(name="tail", bufs=1) as tail, \
         tc.tile_pool(name="ps", bufs=1, space="PSUM") as ps, \
         tc.tile_pool(name="dram", bufs=1, space="DRAM") as dram:

        # ---- input DMA ----
        # Both images ride one concatenated strip tensor, split across the
        # two HWDGE queues (SP and ACT) — DMA trigger instructions cost
        # ~0.7us each on the issuing queue.
        wds = [NB * (P + 2 * w) for w in ws]
        WD = sum(wds)
        st = persist.tile([P, WD], FP16, tag="st")
        for q in range(4):
            eng = nc.sync if q % 2 == 0 else nc.scalar
            eng.dma_start(st[q * 32:(q + 1) * 32, :],
                          srcs[q * 32:(q + 1) * 32, :])

        # ---- collective plumbing ----
        # Per-image collectives: #0 (gt) triggers right after the first
        # column pass — inside the CC start barrier, which absorbs the
        # first-collective launch — doing useful work instead of a dummy
        # warm-up; #1 (pred) queues right behind it. Both are AllGathers
        # (interleaved A/B: as first op ~8-9us vs ~10.5 AllReduce; as
        # second op ~16.2 vs ~19.2) followed by a local 8-max.
        ar_ins = [dram.tile([1, 1], F32, name=f"ari{m}", tag=f"ari{m}")
                  for m in range(2)]
        ar_outs = [nc.dram_tensor("ar_out_sh0", [1, NCORES], F32,
                                  addr_space="Shared"),
                   nc.dram_tensor(
                       "ar_out_sh1",
                       [1, 1 if kind1 == "AllReduce" else NCORES], F32,
                       addr_space="Shared")]

        # ---- constants (DVE is idle while the strips stream in) ----
        io = const.tile([P, P], I32)
        nc.gpsimd.iota(io[:], [[1, P]], base=0, channel_multiplier=-1)
        ident = const.tile([P, P], F32)
        nc.vector.tensor_scalar(ident[:], io[:], 0, None, ALU.is_equal)
        identh = const.tile([P, P], FP16)
        nc.scalar.copy(identh[:], ident[:])
        ones1 = const.tile([1, P], F32)
        nc.vector.memset(ones1[:], 1.0)
        onesh = const.tile([P, WD], FP16)
        nc.vector.memset(onesh[:], 1.0)

        # ============ phases 1+2: row pass + transpose, interleaved ============
        # Per-image row pass on slices of the concatenated strip; image
        # 0's PE transposes + squares run while image 1's scans occupy
        # DVE, so the first column pass starts without a stall. The scans
        # chain across block boundaries, but carried-in state reaches any
        # central column with value > the margin and never wins.
        # foreground -> HINF, background -> 0. Host pre-scales inputs by
        # 1e30 (saturating fp16) so `> 0` is the fg test for both images
        # and fp16 underflow cannot flip tiny positives.
        gTps = []
        gBs = []
        for m in range(2):
            w = ws[m]
            wd = wds[m]
            lo = m * wds[0]
            z = work.tile([P, wd], FP16, tag=f"z{m}")
            nc.vector.tensor_scalar(z[:], st[:, lo:lo + wd], 0.0, HINF,
                                    ALU.is_gt, ALU.mult)
            dl = work.tile([P, wd], FP16, tag=f"dl{m}")
            nc.vector.tensor_tensor_scan(dl[:], onesh[:, :wd], z[:], INF,
                                         ALU.add, ALU.min)
            dr = work.tile([P, wd], FP16, tag=f"dr{m}")
            nc.vector.tensor_tensor_scan(dr[:, ::-1], onesh[:, :wd],
                                         z[:, ::-1], INF, ALU.add, ALU.min)
            g = work.tile([P, wd], FP16, tag=f"g{m}")
            nc.vector.tensor_tensor(g[:], dl[:], dr[:], ALU.min)

            use16 = use16s[m]
            dt = FP16 if use16 else F32
            inf = HINF if use16 else INF
            gw = H + 2 * w
            gTp = persist.tile([P, gw], dt, tag=f"gtp{m}")
            nc.vector.memset(gTp[:, :w], inf)
            nc.vector.memset(gTp[:, w + H:], inf)
            W = P + 2 * w
            for b in range(NB):
                pt = ps.tile([P, P], FP16, tag="pt", bufs=4)
                off = b * W + w
                nc.tensor.transpose(pt[:], g[:, off:off + P], identh[:])
                nc.scalar.activation(gTp[:, w + b * P:w + (b + 1) * P], pt[:],
                                     AF.Square)
            if use16:
                # odd shifts read a one-element-shifted copy so the AP
                # stays 4-byte-aligned for the DVE 2x fp16 mode
                gB = persist.tile([P, gw], FP16, tag=f"gb{m}")
                nc.scalar.copy(gB[:, :gw - 1], gTp[:, 1:])
                nc.vector.memset(gB[:, gw - 1:], inf)
            else:
                gB = None
            gTps.append(gTp)
            gBs.append(gB)

        # ====== phase 3: column min-plus + per-image max + AllReduce ======
        # DVE queue order: col0, mx0, mxr0, col1, mx1, mxr1 — the tiny
        # partition-max of image 0 rides between the passes so AllReduce#0
        # triggers at ~48us, well before the CC barrier ends.
        ys = []
        mxrs = []
        accs = []
        for m in range(2):
            acc = _col_pass(tc, m, ws[m], gTps[m], gBs[m], persist, work)
            accs.append(acc)
            mxp = tail.tile([P, 1], F32, tag=f"mxp{m}")
            nc.vector.reduce_max(mxp[:], acc[:], axis=AX.X)
            pmx = ps.tile([1, P], F32, tag="pmx")
            nc.tensor.transpose(pmx[:], mxp[:], ident[:])
            mxr = tail.tile([1, 1], F32, tag=f"mxr{m}")
            nc.vector.reduce_max(mxr[:], pmx[:], axis=AX.X)
            mxrs.append(mxr)
            nc.sync.dma_start(ar_ins[m][0:1, 0:1], mxr[:])
            if m == 0:
                nc.gpsimd.collective_compute(
                    "AllGather", ALU.bypass, replica_groups=rg,
                    ins=[ar_ins[m][:, :].opt()],
                    outs=[ar_outs[m][:, :].opt()])
            elif kind1 == "AllReduce":
                nc.gpsimd.collective_compute(
                    "AllReduce", ALU.max, replica_groups=rg,
                    ins=[ar_ins[m][:, :].opt()],
                    outs=[ar_outs[m][:, :].opt()])
            else:
                nc.gpsimd.collective_compute(
                    "AllGather", ALU.bypass, replica_groups=rg,
                    ins=[ar_ins[m][:, :].opt()],
                    outs=[ar_outs[m][:, :].opt()])
            # unnormalized distances, precomputed before AllReduce#1 ends
            y = persist.tile([P, H], FP16, tag=f"y{m}")
            nc.scalar.activation(y[:], acc[:], AF.Sqrt)
            ys.append(y)

        # ============ phase 4: per-image inv = 1/(sqrt(max)+1e-6) ============
        # Image 0's chain (and a0 below) runs while AllReduce#1 is in
        # flight; image 1's is the post-collective critical path. The DVE
        # bits are emitted after both column passes so the in-order DVE
        # queue never stalls on a collective result mid-compute.
        def inv_chain(m, guard=None):
            # Fetched on the ACT HWDGE queue: on the SP queue this trigger
            # (gated on the collective's completion) gets scheduled ahead
            # of collective #1's input staging and head-of-line blocks it.
            if m == 1 and kind1 == "AllReduce":
                gmx = tail.tile([1, 1], F32, tag="gmx1")
                nc.scalar.dma_start(gmx[:], ar_outs[1][0:1, 0:1])
            elif m == 1:
                gmx8b = tail.tile([1, NCORES], F32, tag="gmx8b")
                nc.scalar.dma_start(gmx8b[:], ar_outs[1][0:1, :])
                gmx = tail.tile([1, 1], F32, tag="gmx1")
                nc.vector.reduce_max(gmx[:], gmx8b[:], axis=AX.X)
            else:
                gmx8 = tail.tile([1, NCORES], F32, tag=f"gmx8{m}")
                nc.scalar.dma_start(gmx8[:], ar_outs[m][0:1, :])
                gmx = tail.tile([1, 1], F32, tag=f"gmx{m}")
                assert guard is not None
                # The scheduler also hoists DVE ops above earlier-created
                # ones; a collective-gated op hoisted above the image-1
                # column pass would head-of-line block the DVE queue (and
                # with it AllGather#1's trigger). op0=bypass both ignores
                # the `guard` scalar AND records a read of it, pinning
                # this op (and its dependents) behind the column pass.
                t4 = tail.tile([1, 4], F32, tag="t4")
                nc.vector.scalar_tensor_tensor(
                    t4[:], gmx8[:, 0:4], guard, gmx8[:, 4:8],
                    ALU.bypass, ALU.max)
                t2 = tail.tile([1, 2], F32, tag="t2")
                nc.vector.tensor_tensor(t2[:], t4[:, 0:2], t4[:, 2:4],
                                        ALU.max)
                nc.vector.tensor_tensor(gmx[:], t2[:, 0:1], t2[:, 1:2],
                                        ALU.max)
            msq = tail.tile([1, 1], F32, tag=f"msq{m}")
            nc.scalar.activation(msq[:], gmx[:], AF.Sqrt)
            s1 = tail.tile([1, 1], F32, tag=f"s1{m}")
            nc.vector.tensor_scalar_add(s1[:], msq[:], 1e-6)
            nc.vector.reciprocal(s1[:], s1[:])
            pb = ps.tile([P, 1], F32, tag="pb")
            nc.tensor.matmul(pb[:], ones1[:], s1[:])
            invb = tail.tile([P, 1], F32, tag=f"invb{m}")
            nc.scalar.copy(invb[:], pb[:])
            return invb

        # ================= phase 5: normalize + masked mean =================
        # Image 0's inv chain and a0 (on ACT; scale is a per-partition AP)
        # overlap AllReduce#1; image 1's chain and a1 (DVE) are the
        # post-collective critical path.
        invb0 = inv_chain(0, guard=mxrs[1][0:1, 0:1])
        a0 = tail.tile([P, H], FP16, tag="a0")
        nc.scalar.activation(a0[:], ys[0][:], AF.Copy, scale=invb0[:, 0:1])
        invb1 = inv_chain(1)
        a1 = tail.tile([P, H], FP16, tag="a1")
        nc.vector.tensor_scalar(a1[:], ys[1][:], invb1[:, 0:1], None,
                                ALU.mult)
        # (a0 < 0.1) | (a1 < 0.1)  ==  min(a0, a1) < 0.1; the masked-|diff|
        # sum and the mask count ride the ACT accumulators (|mk*df| =
        # mk*|df|), keeping only 5 ops on the post-collective DVE path.
        mk = tail.tile([P, H], FP16, tag="mk")
        nc.vector.tensor_tensor(mk[:], a0[:], a1[:], ALU.min)
        nc.vector.tensor_scalar(mk[:], mk[:], 0.1, None, ALU.is_lt)
        df = tail.tile([P, H], FP16, tag="df")
        nc.vector.tensor_tensor(df[:], a0[:], a1[:], ALU.subtract)
        nc.vector.tensor_tensor(df[:], df[:], mk[:], ALU.mult)
        # Per-partition partial sums go out as [128, 2]; the host finishes
        # the 256-element reduction (frees the PE matmul + PSUM-evacuate
        # hops from the post-collective critical path).
        s12 = tail.tile([P, 2], F32, tag="s12")
        cnt = tail.tile([P, H], FP16, tag="cnt")
        nc.scalar.activation(cnt[:], mk[:], AF.Copy, accum_out=s12[:, 1:2])
        da = tail.tile([P, H], FP16, tag="da")
        nc.scalar.activation(da[:], df[:], AF.Abs, accum_out=s12[:, 0:1])
        nc.sync.dma_start(partials[:, :], s12[:])


def _build(w_gt, w_pred, kind1="AllGather"):
    nc = bacc.Bacc("TRN2", target_bir_lowering=False, debug=False,
                   num_devices=NCORES)
    wd = NB * (P + 2 * w_gt) + NB * (P + 2 * w_pred)
    st = nc.dram_tensor("st", [P, wd], FP16, kind="ExternalInput")
    partials = nc.dram_tensor("partials", [P, 2], F32,
                               kind="ExternalOutput")
    with tile.TileContext(nc) as tc:
        _body(tc, w_gt, w_pred, st, partials, kind1)
    nc.compile()
    return nc


_PROGRAMS = {}


def _program(*key):
    if key not in _PROGRAMS:
        _PROGRAMS[key] = _build(*key)
    return _PROGRAMS[key]


def _row_gmax(fg):
    """Max over pixels of the in-row distance to the nearest background
    pixel (clamped to BIG). This equals the exact column-pass window bound."""
    idx = np.arange(fg.shape[1], dtype=np.float64)
    zero = ~fg
    left = np.maximum.accumulate(np.where(zero, idx, -np.inf), axis=1)
    right = np.minimum.accumulate(np.where(zero, idx, np.inf)[:, ::-1],
                                  axis=1)[:, ::-1]
    g = np.minimum(np.minimum(idx - left, right - idx), BIG)
    return float(g.max())


def _bucket(gmax):
    need = min(int(np.ceil(gmax)), H - 1)
    for b in _BUCKETS:
        if b >= need:
            return b
    raise NotImplementedError(
        f"row gmax {gmax} exceeds the supported strip margin {_BUCKETS[-1]}")


def _strips(img, w):
    """Per-core fp16 strips [128, 8*(128+2w)]: strip[c][p, b*(128+2w)+q] =
    scaled img[128*b + p, 128*c - w + q], fg-padded outside the image."""
    x = np.asarray(img, np.float32) * 1e30
    pad = np.full((H, w), np.float32(1e30))
    xp = np.concatenate([pad, x, pad], axis=1)
    W = P + 2 * w
    out = []
    for c in range(NCORES):
        b = xp[:, c * P:c * P + W].astype(np.float16)
        out.append(np.ascontiguousarray(
            b.reshape(NB, P, W).transpose(1, 0, 2).reshape(P, NB * W)))
    return out


def _run(pred, gt, trace=False):
    pred = np.ascontiguousarray(np.asarray(pred), dtype=np.float32)
    gt = np.ascontiguousarray(np.asarray(gt), dtype=np.float32)
    assert pred.shape == (H, H) and gt.shape == (H, H)
    w_gt = _bucket(_row_gmax(gt != 0))
    w_pred = _bucket(_row_gmax(pred > 0))
    nc = _program(w_gt, w_pred)
    sg = _strips(gt, w_gt)
    sp = _strips(pred, w_pred)
    in_maps = [{"st": np.ascontiguousarray(
        np.concatenate([sg[c], sp[c]], axis=1))} for c in range(NCORES)]
    res = run_bass_kernel_spmd(nc, in_maps, list(range(NCORES)), trace=trace)
    tot = np.zeros(2, np.float64)
    for r in res.results:
        tot += np.asarray(r["partials"], np.float64).reshape(P, 2).sum(0)
    loss = np.float32(tot[0] / max(tot[1], 1.0))
    return loss, res


def kernel(pred, gt):
    loss, _ = _run(pred, gt)
    return loss
